# revision 33
# baseline (speedup 1.0000x reference)
"""Trainium2 Bass kernel for hierarchical (sibling-group) softmax over
hyperplane margins.

For x:(8,64,128,128), normals/offsets:(1024,64), sibmat block-diagonal with
32-wide sibling groups:

    logits[b,m,h,w] = <x[b,:,h,w], normals[m]> - <normals[m], offsets[m]>
    out = exp(logits) / (group_sum_32(exp(logits)) + 1e-15)

Sharding: data-parallel over batch, one batch element per NeuronCore (8
cores), no collectives.  Per core, m-chunks of 128 rows live on partitions
and pixels on the free axis, in blocks of 1024 pixels:

  1. PE matmul (fp16 in, fp32 psum): logits = normals.T @ x.
  2. ACT exp PSUM->SBUF with the per-partition bias argument supplying
     -<normals_m, offsets_m> exactly in fp32.
  3. PE matmul with a [128, 32] group-indicator accumulates the 32-wide
     group sums of four pixel blocks into one base-aligned [128, 1024]
     PSUM tile (rows 32*bg + group).
  4. ACT copies Z to SBUF (shares the exp ACT table), one DVE reciprocal
     per 4-block super-group (the DVE reciprocal is ~6.3 cyc/elem, so it
     must be amortized), output in fp16.
  5. PE matmul with a [32, 128] indicator broadcasts 1/Z rows back onto
     the chunk's 128 partitions (PSUM), and one DVE multiply per half
     produces the fp32 output tile.
  6. 512KB output DMAs land directly in the reference (M, H, W) layout.

Phase 5/6 of each super-group is software-pipelined ~1.4 groups behind
phases 1-3 so the reciprocal latency hides under PE work.  A post-pass
splits multi-wait instructions (walrus's TRN2 codegen encodes at most one
semaphore wait per compute instruction).  Matmul operands are fp16 because
fp32 matmuls stream at 4 cyc/row and the PE clock holds at 1.2 GHz in this
dependency pattern; all value ranges here are O(1) so fp16's 10-bit
mantissa keeps the output within ~1.5e-3 of the fp32 reference (guarded by
input-range checks that fall back to an exact host implementation).
"""

import numpy as np

B, D, H, W = 8, 64, 128, 128
M = 1024
GROUP = 32
PIX = H * W          # 16384 pixels per batch element
BLK = 1024           # pixels per block
NBLK = PIX // BLK    # 16
MC = 128             # m-chunk width (partition dim)
NCHUNK = M // MC     # 8
KAUG = D + 1         # contraction dim augmented with the bias row
NCORES = 8
FMAX = 512           # max moving free dim for fp32 matmul

_cache = {}


_WAIT_OK_OPCODES = {"Call"}


def _split_excess_waits(nc):
    """Walrus's TRN2 codegen (CoreV3GenImpl setupSyncWait) encodes at most
    one semaphore wait per compute instruction (Matmult, TensorTensor, ...);
    Tile can legitimately attach several (e.g. waits on two input DMAs).
    Move all but one wait onto EventSemaphore instructions inserted just
    before the instruction on the same engine — ordering is identical."""
    import concourse.mybir as mybir

    n_fixed = 0
    for f in nc.m.functions:
        for blk in f.blocks:
            out = []
            changed = False
            for inst in blk.instructions:
                si = inst.sync_info
                if (
                    si is not None
                    and len(si.on_wait) > 1
                    and inst.opcode not in _WAIT_OK_OPCODES
                ):
                    waits = list(si.on_wait)
                    for j, w in enumerate(waits[:-1]):
                        out.append(
                            mybir.InstEventSemaphore(
                                name=f"{inst.name}-wsplit{j}",
                                opcode="EventSemaphore",
                                engine=inst.engine,
                                sync_info=mybir.SyncInfo(
                                    on_wait=[w], on_update=[]
                                ),
                            )
                        )
                    inst.sync_info = mybir.SyncInfo(
                        on_wait=[waits[-1]], on_update=list(si.on_update)
                    )
                    changed = True
                    n_fixed += 1
                out.append(inst)
            if changed:
                blk.instructions = out
    return n_fixed


def _build_nc(pix=PIX, split_waits=True):
    import concourse.bass as bass
    import concourse.mybir as mybir
    import concourse.tile as tile

    f32 = mybir.dt.float32
    f16 = mybir.dt.float16
    nblk = pix // BLK
    ngrp = M // GROUP        # 32 groups total
    cgrp = MC // GROUP       # 4 groups per m-chunk
    GB = 4                   # pixel-blocks per Z-accumulation super-group
    nc = bass.Bass()
    # fp16 matmul chain: PE streams 1 cyc/row (fp32 is 4 cyc) and with this
    # problem's O(1) value ranges fp16's 10-bit mantissa keeps the output
    # within ~1e-3 relative of the fp32 reference.
    x_in = nc.declare_dram_parameter("x_bf", [D, pix], f16, isOutput=False)
    w_in = nc.declare_dram_parameter("normals_bf", [D, M], f16, isOutput=False)
    # gsum_w[:, mc*32:(mc+1)*32]: [128, 32] indicator, [p, r] = 1 iff
    # r == 8*(p//32) + replica; row r of the product = group (p//32) sum,
    # replicated so mm3 can select it from any 32-aligned base.
    g_in = nc.declare_dram_parameter("gsum_w", [MC, NCHUNK * ngrp], f16,
                                     isOutput=False)
    # gbc_w[32*q + r, mc*128 + p]: replica q of the [32, 128] indicator that
    # broadcasts group row r onto chunk partitions p; replicas let mm3's
    # stationary operand start at the same partition base as its moving
    # operand (rows 32*bg of the shared Z tile).
    b_in = nc.declare_dram_parameter("gbc_w", [GB * ngrp, NCHUNK * MC], f16,
                                     isOutput=False)
    # bias_neg[p, mc] = -<normals, offsets> for m = mc*128+p; applied as the
    # ACT exp per-partition bias (exact fp32, frees the K=65 aug row).
    c_in = nc.declare_dram_parameter("bias_neg", [MC, NCHUNK], f32,
                                     isOutput=False)
    y_out = nc.declare_dram_parameter("y", [M, pix], f32, isOutput=True)

    with tile.TileContext(nc) as tc:
        with (
            tc.tile_pool(name="const", bufs=1) as cpool,
            tc.tile_pool(name="xin", bufs=3) as xpool,
            tc.tile_pool(name="expv", bufs=GB * NCHUNK + 20) as epool,
            tc.tile_pool(name="zsb", bufs=2) as zpool,
            tc.tile_pool(name="recv", bufs=2) as rpool,
            tc.tile_pool(name="outv", bufs=4) as opool,
            tc.tile_pool(name="psl", bufs=2, space="PSUM") as psl,
            tc.tile_pool(name="psz", bufs=1, space="PSUM") as psz,
            tc.tile_pool(name="psb", bufs=2, space="PSUM") as psb,
        ):
            w_sb = cpool.tile([D, M], f16)
            nc.sync.dma_start(w_sb[:], w_in[:])
            g_sb = cpool.tile([MC, NCHUNK * ngrp], f16)
            nc.sync.dma_start(g_sb[:], g_in[:])
            b_sb = cpool.tile([GB * ngrp, NCHUNK * MC], f16)
            nc.sync.dma_start(b_sb[:], b_in[:])
            c_sb = cpool.tile([MC, NCHUNK], f32)
            nc.sync.dma_start(c_sb[:], c_in[:])

            ngroups = nblk // GB
            e_all = {}
            rec_of = {}

            def phase1(g, bg, mc):
                blk = g * GB + bg
                if mc == 0:
                    x_t = xpool.tile([D, BLK], f16, tag="x_t", name="x_t")
                    nc.sync.dma_start(
                        x_t[:], x_in[:, blk * BLK:(blk + 1) * BLK]
                    )
                    e_all["x", g, bg] = x_t
                x_t = e_all["x", g, bg]
                ps_l = psl.tile([MC, BLK], f32, tag="ps_l", name="ps_l")
                for h in range(BLK // FMAX):
                    nc.tensor.matmul(
                        ps_l[:, h * FMAX:(h + 1) * FMAX],
                        w_sb[:, mc * MC:(mc + 1) * MC],
                        x_t[:, h * FMAX:(h + 1) * FMAX],
                        start=True, stop=True,
                    )
                e_t = epool.tile([MC, BLK], f16, tag="e_t", name="e_t")
                nc.scalar.activation(
                    e_t[:], ps_l[:],
                    mybir.ActivationFunctionType.Exp,
                    bias=c_sb[:, mc:mc + 1],
                )
                e_all[g, bg, mc] = e_t
                ps_z = e_all["z", g]
                for h in range(BLK // FMAX):
                    nc.tensor.matmul(
                        ps_z[bg * ngrp:(bg + 1) * ngrp,
                             h * FMAX:(h + 1) * FMAX],
                        g_sb[:, mc * ngrp:(mc + 1) * ngrp],
                        e_t[:, h * FMAX:(h + 1) * FMAX],
                        start=(mc == 0), stop=(mc == NCHUNK - 1),
                        tile_position=(0, bg * ngrp),
                    )

            def finish_z(g):
                # free the psz tile quickly (ACT copy shares the exp table),
                # then the slow DVE reciprocal runs off SBUF.
                ps_z = e_all.pop(("z", g))
                z_sb = zpool.tile([GB * ngrp, BLK], f32, tag="z_sb", name="z_sb")
                cp = nc.scalar.copy(z_sb[:], ps_z[:])
                rec = rpool.tile([GB * ngrp, BLK], f16, tag="rec", name="rec")
                with nc.allow_low_precision(
                    reason="fp16 rounding of reciprocal feeding the "
                    "broadcast matmul; well within output tolerance"
                ):
                    rc_i = nc.vector.reciprocal(rec[:], z_sb[:])
                # hoist the rec chain ahead of queued multiplies: the whole
                # next super-group's broadcasts depend on it
                for inst in (cp, rc_i):
                    try:
                        inst.inst.bass_priority = -1000
                    except AttributeError:
                        try:
                            inst.bass_priority = -1000
                        except AttributeError:
                            pass
                rec_of[g] = rec

            def phase2(g, bg, mc):
                blk = g * GB + bg
                rec = rec_of[g]
                e_t = e_all.pop((g, bg, mc))
                o_t = opool.tile([MC, BLK], f32, tag="o_t", name="o_t")
                for h in range(BLK // FMAX):
                    ps_b = psb.tile([MC, FMAX], f32, tag="ps_b", name="ps_b")
                    nc.tensor.matmul(
                        ps_b[:],
                        b_sb[bg * ngrp:(bg + 1) * ngrp,
                             mc * MC:(mc + 1) * MC],
                        rec[bg * ngrp:(bg + 1) * ngrp,
                            h * FMAX:(h + 1) * FMAX],
                        start=True, stop=True,
                        tile_position=(bg * ngrp, 0),
                    )
                    nc.vector.tensor_mul(
                        o_t[:, h * FMAX:(h + 1) * FMAX],
                        e_t[:, h * FMAX:(h + 1) * FMAX],
                        ps_b[:],
                    )
                nc.sync.dma_start(
                    y_out[mc * MC:(mc + 1) * MC,
                          blk * BLK:(blk + 1) * BLK],
                    o_t[:],
                )

            # Software pipeline: phase2 lags phase1 by one full group plus
            # LAG chunks, so the copy+reciprocal chain finishes while the PE
            # streams the next group's matmuls instead of stalling on it.
            CPG = GB * NCHUNK            # chunks per group
            LAG = 12
            nunits = ngroups * CPG

            def unit(j):
                return j // CPG, (j // NCHUNK) % GB, j % NCHUNK

            for j in range(nunits + CPG + LAG):
                if j < nunits:
                    g, bg, mc = unit(j)
                    if bg == 0 and mc == 0:
                        e_all["z", g] = psz.tile(
                            [GB * ngrp, BLK], f32, tag="ps_z", name="ps_z"
                        )
                    phase1(g, bg, mc)
                    if bg == GB - 1 and mc == NCHUNK - 1:
                        finish_z(g)
                k = j - CPG - LAG
                if k >= 0:
                    phase2(*unit(k))
    if split_waits:
        _split_excess_waits(nc)
    return nc


def _prep_core_inputs(x, normals, offsets, sibmat):
    bf16 = np.float16
    bias = np.einsum("md,md->m", normals, offsets).astype(np.float32)
    w_bf = np.ascontiguousarray(normals.T).astype(bf16)

    ngrp = M // GROUP
    gid = np.arange(M) // GROUP                     # global group of each m
    gsum = np.zeros((MC, NCHUNK * ngrp), np.float32)
    gbc = np.zeros((ngrp, NCHUNK * MC), np.float32)
    for mc in range(NCHUNK):
        for p in range(MC):
            r = gid[mc * MC + p]
            gsum[p, mc * ngrp + r] = 1.0
            gbc[r, mc * MC + p] = 1.0
    gsum = gsum.astype(bf16)
    gbc = np.tile(gbc, (4, 1)).astype(bf16)
    bias_neg = np.ascontiguousarray(
        -bias.reshape(NCHUNK, MC).T
    ).astype(np.float32)

    in_maps = []
    for b in range(NCORES):
        x_bf = np.ascontiguousarray(x[b].reshape(D, PIX)).astype(bf16)
        in_maps.append(
            {"x_bf": x_bf, "normals_bf": w_bf, "gsum_w": gsum,
             "gbc_w": gbc, "bias_neg": bias_neg}
        )
    return in_maps


def _sibmat_is_expected(sibmat):
    gid = np.arange(M) // GROUP
    expected = (gid[:, None] == gid[None, :]).astype(np.float32)
    return sibmat.shape == (M, M) and np.array_equal(sibmat, expected)


def _numpy_fallback(x, normals, offsets, sibmat):
    # Straight fp32 transcription of the reference; only used if sibmat is
    # not the expected 32-wide block-diagonal matrix.
    bias = np.einsum("md,md->m", normals, offsets)
    out = np.empty((B, M, H, W), np.float32)
    for b in range(B):
        logits = np.einsum("dhw,md->mhw", x[b], normals) - bias[:, None, None]
        logits -= np.max(logits, axis=0, keepdims=True)
        e = np.exp(logits)
        z = np.einsum("mhw,nm->nhw", e, sibmat)
        out[b] = e / (z + 1e-15)
    return out


def kernel(x, normals, offsets, sibmat, steps=None, trace=False, **_ignored):
    from concourse.bass_utils import run_bass_kernel_spmd


    x = np.ascontiguousarray(np.asarray(x, dtype=np.float32))
    normals = np.ascontiguousarray(np.asarray(normals, dtype=np.float32))
    offsets = np.ascontiguousarray(np.asarray(offsets, dtype=np.float32))
    sibmat = np.ascontiguousarray(np.asarray(sibmat, dtype=np.float32))

    if (
        not _sibmat_is_expected(sibmat)
        or np.abs(normals).max() > 0.5
        or np.abs(x).max() > 12.0
    ):
        # unexpected structure or value ranges outside the fp16-safe
        # envelope of the device kernel: compute exactly on host
        return _numpy_fallback(x, normals, offsets, sibmat)

    if "nc" not in _cache:
        _cache["nc"] = _build_nc()
    nc = _cache["nc"]

    in_maps = _prep_core_inputs(x, normals, offsets, sibmat)
    res = run_bass_kernel_spmd(nc, in_maps, list(range(NCORES)), trace=trace)
    out = np.stack(
        [np.asarray(r["y"]).reshape(M, H, W) for r in res.results]
    )
    kernel.last_result = res
    return out


# revision 37
# speedup vs baseline: 1.0193x; 1.0193x over previous
"""Trainium2 Bass kernel for hierarchical (sibling-group) softmax over
hyperplane margins.

For x:(8,64,128,128), normals/offsets:(1024,64), sibmat block-diagonal with
32-wide sibling groups:

    logits[b,m,h,w] = <x[b,:,h,w], normals[m]> - <normals[m], offsets[m]>
    out = exp(logits) / (group_sum_32(exp(logits)) + 1e-15)

Sharding: data-parallel over batch, one batch element per NeuronCore (8
cores), no collectives.  Per core, m-chunks of 128 rows live on partitions
and pixels on the free axis, in blocks of 1024 pixels:

  1. PE matmul (fp16 in, fp32 psum): logits = normals.T @ x.
  2. ACT exp PSUM->SBUF with the per-partition bias argument supplying
     -<normals_m, offsets_m> exactly in fp32.
  3. PE matmul with a [128, 32] group-indicator accumulates the 32-wide
     group sums of four pixel blocks into one base-aligned [128, 1024]
     PSUM tile (rows 32*bg + group).
  4. ACT copies Z to SBUF (shares the exp ACT table), one DVE reciprocal
     per 4-block super-group (the DVE reciprocal is ~6.3 cyc/elem, so it
     must be amortized), output in fp16.
  5. PE matmul with a [32, 128] indicator broadcasts 1/Z rows back onto
     the chunk's 128 partitions (PSUM), and one DVE multiply per half
     produces the fp32 output tile.
  6. 512KB output DMAs land directly in the reference (M, H, W) layout.

Phase 5/6 of each super-group is software-pipelined ~1.4 groups behind
phases 1-3 so the reciprocal latency hides under PE work.  A post-pass
splits multi-wait instructions (walrus's TRN2 codegen encodes at most one
semaphore wait per compute instruction).  Matmul operands are fp16 because
fp32 matmuls stream at 4 cyc/row and the PE clock holds at 1.2 GHz in this
dependency pattern; all value ranges here are O(1) so fp16's 10-bit
mantissa keeps the output within ~1.5e-3 of the fp32 reference (guarded by
input-range checks that fall back to an exact host implementation).
"""

import numpy as np

B, D, H, W = 8, 64, 128, 128
M = 1024
GROUP = 32
PIX = H * W          # 16384 pixels per batch element
BLK = 1024           # pixels per block
NBLK = PIX // BLK    # 16
MC = 128             # m-chunk width (partition dim)
NCHUNK = M // MC     # 8
KAUG = D + 1         # contraction dim augmented with the bias row
NCORES = 8
FMAX = 512           # max moving free dim for fp32 matmul

_cache = {}


_WAIT_OK_OPCODES = {"Call"}


def _split_excess_waits(nc):
    """Walrus's TRN2 codegen (CoreV3GenImpl setupSyncWait) encodes at most
    one semaphore wait per compute instruction (Matmult, TensorTensor, ...);
    Tile can legitimately attach several (e.g. waits on two input DMAs).
    Move all but one wait onto EventSemaphore instructions inserted just
    before the instruction on the same engine — ordering is identical."""
    import concourse.mybir as mybir

    n_fixed = 0
    for f in nc.m.functions:
        for blk in f.blocks:
            out = []
            changed = False
            for inst in blk.instructions:
                si = inst.sync_info
                if (
                    si is not None
                    and len(si.on_wait) > 1
                    and inst.opcode not in _WAIT_OK_OPCODES
                ):
                    waits = list(si.on_wait)
                    for j, w in enumerate(waits[:-1]):
                        out.append(
                            mybir.InstEventSemaphore(
                                name=f"{inst.name}-wsplit{j}",
                                opcode="EventSemaphore",
                                engine=inst.engine,
                                sync_info=mybir.SyncInfo(
                                    on_wait=[w], on_update=[]
                                ),
                            )
                        )
                    inst.sync_info = mybir.SyncInfo(
                        on_wait=[waits[-1]], on_update=list(si.on_update)
                    )
                    changed = True
                    n_fixed += 1
                out.append(inst)
            if changed:
                blk.instructions = out
    return n_fixed


def _build_nc(pix=PIX, split_waits=True):
    import concourse.bass as bass
    import concourse.mybir as mybir
    import concourse.tile as tile

    f32 = mybir.dt.float32
    f16 = mybir.dt.float16
    nblk = pix // BLK
    ngrp = M // GROUP        # 32 groups total
    cgrp = MC // GROUP       # 4 groups per m-chunk
    GB = 4                   # pixel-blocks per Z-accumulation super-group
    nc = bass.Bass()
    # fp16 matmul chain: PE streams 1 cyc/row (fp32 is 4 cyc) and with this
    # problem's O(1) value ranges fp16's 10-bit mantissa keeps the output
    # within ~1e-3 relative of the fp32 reference.
    x_in = nc.declare_dram_parameter("x_bf", [D, pix], f16, isOutput=False)
    w_in = nc.declare_dram_parameter("normals_bf", [D, M], f16, isOutput=False)
    # gsum_w[:, mc*32:(mc+1)*32]: [128, 32] indicator, [p, r] = 1 iff
    # r == 8*(p//32) + replica; row r of the product = group (p//32) sum,
    # replicated so mm3 can select it from any 32-aligned base.
    g_in = nc.declare_dram_parameter("gsum_w", [MC, NCHUNK * ngrp], f16,
                                     isOutput=False)
    # gbc_w[32*q + r, mc*128 + p]: replica q of the [32, 128] indicator that
    # broadcasts group row r onto chunk partitions p; replicas let mm3's
    # stationary operand start at the same partition base as its moving
    # operand (rows 32*bg of the shared Z tile).
    b_in = nc.declare_dram_parameter("gbc_w", [GB * ngrp, NCHUNK * MC], f16,
                                     isOutput=False)
    # bias_neg[p, mc] = -<normals, offsets> for m = mc*128+p; applied as the
    # ACT exp per-partition bias (exact fp32, frees the K=65 aug row).
    c_in = nc.declare_dram_parameter("bias_neg", [MC, NCHUNK], f32,
                                     isOutput=False)
    y_out = nc.declare_dram_parameter("y", [M, pix], f32, isOutput=True)

    with tile.TileContext(nc) as tc:
        with (
            tc.tile_pool(name="const", bufs=1) as cpool,
            tc.tile_pool(name="xin", bufs=3) as xpool,
            tc.tile_pool(name="expv", bufs=GB * NCHUNK + 20) as epool,
            tc.tile_pool(name="zsb", bufs=2) as zpool,
            tc.tile_pool(name="recv", bufs=2) as rpool,
            tc.tile_pool(name="outv", bufs=4) as opool,
            tc.tile_pool(name="psl", bufs=2, space="PSUM") as psl,
            tc.tile_pool(name="psz", bufs=1, space="PSUM") as psz,
            tc.tile_pool(name="psb", bufs=2, space="PSUM") as psb,
        ):
            w_sb = cpool.tile([D, M], f16)
            nc.sync.dma_start(w_sb[:], w_in[:])
            g_sb = cpool.tile([MC, NCHUNK * ngrp], f16)
            nc.sync.dma_start(g_sb[:], g_in[:])
            b_sb = cpool.tile([GB * ngrp, NCHUNK * MC], f16)
            nc.sync.dma_start(b_sb[:], b_in[:])
            c_sb = cpool.tile([MC, NCHUNK], f32)
            nc.sync.dma_start(c_sb[:], c_in[:])

            ngroups = nblk // GB
            e_all = {}
            rec_of = {}

            def phase1(g, bg, mc):
                blk = g * GB + bg
                if mc == 0:
                    x_t = xpool.tile([D, BLK], f16, tag="x_t", name="x_t")
                    nc.sync.dma_start(
                        x_t[:], x_in[:, blk * BLK:(blk + 1) * BLK]
                    )
                    e_all["x", g, bg] = x_t
                x_t = e_all["x", g, bg]
                ps_l = psl.tile([MC, BLK], f32, tag="ps_l", name="ps_l")
                for h in range(BLK // FMAX):
                    nc.tensor.matmul(
                        ps_l[:, h * FMAX:(h + 1) * FMAX],
                        w_sb[:, mc * MC:(mc + 1) * MC],
                        x_t[:, h * FMAX:(h + 1) * FMAX],
                        start=True, stop=True,
                    )
                e_t = epool.tile([MC, BLK], f16, tag="e_t", name="e_t")
                nc.scalar.activation(
                    e_t[:], ps_l[:],
                    mybir.ActivationFunctionType.Exp,
                    bias=c_sb[:, mc:mc + 1],
                )
                e_all[g, bg, mc] = e_t
                ps_z = e_all["z", g]
                for h in range(BLK // FMAX):
                    nc.tensor.matmul(
                        ps_z[bg * ngrp:(bg + 1) * ngrp,
                             h * FMAX:(h + 1) * FMAX],
                        g_sb[:, mc * ngrp:(mc + 1) * ngrp],
                        e_t[:, h * FMAX:(h + 1) * FMAX],
                        start=(mc == 0), stop=(mc == NCHUNK - 1),
                        tile_position=(0, bg * ngrp),
                    )

            z_of = {}

            def copy_z(g):
                # free the psz tile quickly (ACT copy shares the exp table)
                # so the next group's Z accumulation can start
                ps_z = e_all.pop(("z", g))
                z_sb = zpool.tile([GB * ngrp, BLK], f32, tag="z_sb", name="z_sb")
                nc.scalar.copy(z_sb[:], ps_z[:])
                z_of[g] = z_sb

            def emit_recip(g, q):
                # emitted later, as column-quarters spaced across units, so
                # the in-order DVE stream interleaves multiplies between the
                # reciprocal pieces instead of stalling 6.5us behind one
                # monolithic instruction (reciprocal cost is free-dim driven,
                # so column splits keep the total constant)
                QW = BLK // 4
                if q == 0:
                    z_of[g, "rec"] = rpool.tile(
                        [GB * ngrp, BLK], f16, tag="rec", name="rec"
                    )
                rec = z_of[g, "rec"]
                z_sb = z_of[g]
                with nc.allow_low_precision(
                    reason="fp16 rounding of reciprocal feeding the "
                    "broadcast matmul; well within output tolerance"
                ):
                    nc.vector.reciprocal(
                        rec[:, q * QW:(q + 1) * QW],
                        z_sb[:, q * QW:(q + 1) * QW],
                    )
                if q == 3:
                    del z_of[g]
                    rec_of[g] = z_of.pop((g, "rec"))

            def phase2(g, bg, mc):
                blk = g * GB + bg
                rec = rec_of[g]
                e_t = e_all.pop((g, bg, mc))
                o_t = opool.tile([MC, BLK], f32, tag="o_t", name="o_t")
                for h in range(BLK // FMAX):
                    ps_b = psb.tile([MC, FMAX], f32, tag="ps_b", name="ps_b")
                    nc.tensor.matmul(
                        ps_b[:],
                        b_sb[bg * ngrp:(bg + 1) * ngrp,
                             mc * MC:(mc + 1) * MC],
                        rec[bg * ngrp:(bg + 1) * ngrp,
                            h * FMAX:(h + 1) * FMAX],
                        start=True, stop=True,
                        tile_position=(bg * ngrp, 0),
                    )
                    nc.vector.tensor_mul(
                        o_t[:, h * FMAX:(h + 1) * FMAX],
                        e_t[:, h * FMAX:(h + 1) * FMAX],
                        ps_b[:],
                    )
                nc.sync.dma_start(
                    y_out[mc * MC:(mc + 1) * MC,
                          blk * BLK:(blk + 1) * BLK],
                    o_t[:],
                )

            # Software pipeline: phase2 lags phase1 by one full group plus
            # LAG chunks, so the copy+reciprocal chain finishes while the PE
            # streams the next group's matmuls instead of stalling on it.
            CPG = GB * NCHUNK            # chunks per group
            LAG = 14
            FZLAG = 6
            nunits = ngroups * CPG
            recip_at = {}

            def unit(j):
                return j // CPG, (j // NCHUNK) % GB, j % NCHUNK

            for j in range(nunits + CPG + LAG):
                if j < nunits:
                    g, bg, mc = unit(j)
                    if bg == 0 and mc == 0:
                        e_all["z", g] = psz.tile(
                            [GB * ngrp, BLK], f32, tag="ps_z", name="ps_z"
                        )
                    phase1(g, bg, mc)
                    if bg == GB - 1 and mc == NCHUNK - 1:
                        copy_z(g)
                        for q in range(4):
                            recip_at[j + FZLAG + 2 * q] = (g, q)
                if j in recip_at:
                    emit_recip(*recip_at.pop(j))
                k = j - CPG - LAG
                if k >= 0:
                    phase2(*unit(k))
    if split_waits:
        _split_excess_waits(nc)
    return nc


def _prep_core_inputs(x, normals, offsets, sibmat):
    bf16 = np.float16
    bias = np.einsum("md,md->m", normals, offsets).astype(np.float32)
    w_bf = np.ascontiguousarray(normals.T).astype(bf16)

    ngrp = M // GROUP
    gid = np.arange(M) // GROUP                     # global group of each m
    gsum = np.zeros((MC, NCHUNK * ngrp), np.float32)
    gbc = np.zeros((ngrp, NCHUNK * MC), np.float32)
    for mc in range(NCHUNK):
        for p in range(MC):
            r = gid[mc * MC + p]
            gsum[p, mc * ngrp + r] = 1.0
            gbc[r, mc * MC + p] = 1.0
    gsum = gsum.astype(bf16)
    gbc = np.tile(gbc, (4, 1)).astype(bf16)
    bias_neg = np.ascontiguousarray(
        -bias.reshape(NCHUNK, MC).T
    ).astype(np.float32)

    in_maps = []
    for b in range(NCORES):
        x_bf = np.ascontiguousarray(x[b].reshape(D, PIX)).astype(bf16)
        in_maps.append(
            {"x_bf": x_bf, "normals_bf": w_bf, "gsum_w": gsum,
             "gbc_w": gbc, "bias_neg": bias_neg}
        )
    return in_maps


def _sibmat_is_expected(sibmat):
    gid = np.arange(M) // GROUP
    expected = (gid[:, None] == gid[None, :]).astype(np.float32)
    return sibmat.shape == (M, M) and np.array_equal(sibmat, expected)


def _numpy_fallback(x, normals, offsets, sibmat):
    # Straight fp32 transcription of the reference; only used if sibmat is
    # not the expected 32-wide block-diagonal matrix.
    bias = np.einsum("md,md->m", normals, offsets)
    out = np.empty((B, M, H, W), np.float32)
    for b in range(B):
        logits = np.einsum("dhw,md->mhw", x[b], normals) - bias[:, None, None]
        logits -= np.max(logits, axis=0, keepdims=True)
        e = np.exp(logits)
        z = np.einsum("mhw,nm->nhw", e, sibmat)
        out[b] = e / (z + 1e-15)
    return out


def kernel(x, normals, offsets, sibmat, steps=None, trace=False, **_ignored):
    from concourse.bass_utils import run_bass_kernel_spmd


    x = np.ascontiguousarray(np.asarray(x, dtype=np.float32))
    normals = np.ascontiguousarray(np.asarray(normals, dtype=np.float32))
    offsets = np.ascontiguousarray(np.asarray(offsets, dtype=np.float32))
    sibmat = np.ascontiguousarray(np.asarray(sibmat, dtype=np.float32))

    if (
        not _sibmat_is_expected(sibmat)
        or np.abs(normals).max() > 0.5
        or np.abs(x).max() > 12.0
    ):
        # unexpected structure or value ranges outside the fp16-safe
        # envelope of the device kernel: compute exactly on host
        return _numpy_fallback(x, normals, offsets, sibmat)

    if "nc" not in _cache:
        _cache["nc"] = _build_nc()
    nc = _cache["nc"]

    in_maps = _prep_core_inputs(x, normals, offsets, sibmat)
    res = run_bass_kernel_spmd(nc, in_maps, list(range(NCORES)), trace=trace)
    out = np.stack(
        [np.asarray(r["y"]).reshape(M, H, W) for r in res.results]
    )
    kernel.last_result = res
    return out


# revision 38
# speedup vs baseline: 1.2004x; 1.1777x over previous
"""Trainium2 Bass kernel for hierarchical (sibling-group) softmax over
hyperplane margins.

For x:(8,64,128,128), normals/offsets:(1024,64), sibmat block-diagonal with
32-wide sibling groups:

    logits[b,m,h,w] = <x[b,:,h,w], normals[m]> - <normals[m], offsets[m]>
    out = exp(logits) / (group_sum_32(exp(logits)) + 1e-15)

Sharding: data-parallel over batch, one batch element per NeuronCore (8
cores), no collectives.  Per core, m-chunks of 128 rows live on partitions
and pixels on the free axis, in blocks of 1024 pixels:

  1. PE matmul (fp16 in, fp32 psum): logits = normals.T @ x.
  2. ACT exp PSUM->SBUF with the per-partition bias argument supplying
     -<normals_m, offsets_m> exactly in fp32.
  3. PE matmul with a [128, 32] group-indicator accumulates the 32-wide
     group sums of four pixel blocks into one base-aligned [128, 1024]
     PSUM tile (rows 32*bg + group).
  4. ACT copies Z to SBUF (shares the exp ACT table), one DVE reciprocal
     per 4-block super-group (the DVE reciprocal is ~6.3 cyc/elem, so it
     must be amortized), output in fp16.
  5. PE matmul with a [32, 128] indicator broadcasts 1/Z rows back onto
     the chunk's 128 partitions (PSUM), and one DVE multiply per half
     produces the fp32 output tile.
  6. 512KB output DMAs land directly in the reference (M, H, W) layout.

Phase 5/6 of each super-group is software-pipelined ~1.4 groups behind
phases 1-3 so the reciprocal latency hides under PE work.  A post-pass
splits multi-wait instructions (walrus's TRN2 codegen encodes at most one
semaphore wait per compute instruction).  Matmul operands are fp16 because
fp32 matmuls stream at 4 cyc/row and the PE clock holds at 1.2 GHz in this
dependency pattern; all value ranges here are O(1) so fp16's 10-bit
mantissa keeps the output within ~1.5e-3 of the fp32 reference (guarded by
input-range checks that fall back to an exact host implementation).
"""

import numpy as np

B, D, H, W = 8, 64, 128, 128
M = 1024
GROUP = 32
PIX = H * W          # 16384 pixels per batch element
BLK = 1024           # pixels per block
NBLK = PIX // BLK    # 16
MC = 128             # m-chunk width (partition dim)
NCHUNK = M // MC     # 8
KAUG = D + 1         # contraction dim augmented with the bias row
NCORES = 8
FMAX = 512           # max moving free dim for fp32 matmul

_cache = {}


_WAIT_OK_OPCODES = {"Call"}


def _split_excess_waits(nc):
    """Walrus's TRN2 codegen (CoreV3GenImpl setupSyncWait) encodes at most
    one semaphore wait per compute instruction (Matmult, TensorTensor, ...);
    Tile can legitimately attach several (e.g. waits on two input DMAs).
    Move all but one wait onto EventSemaphore instructions inserted just
    before the instruction on the same engine — ordering is identical."""
    import concourse.mybir as mybir

    n_fixed = 0
    for f in nc.m.functions:
        for blk in f.blocks:
            out = []
            changed = False
            for inst in blk.instructions:
                si = inst.sync_info
                if (
                    si is not None
                    and len(si.on_wait) > 1
                    and inst.opcode not in _WAIT_OK_OPCODES
                ):
                    waits = list(si.on_wait)
                    for j, w in enumerate(waits[:-1]):
                        out.append(
                            mybir.InstEventSemaphore(
                                name=f"{inst.name}-wsplit{j}",
                                opcode="EventSemaphore",
                                engine=inst.engine,
                                sync_info=mybir.SyncInfo(
                                    on_wait=[w], on_update=[]
                                ),
                            )
                        )
                    inst.sync_info = mybir.SyncInfo(
                        on_wait=[waits[-1]], on_update=list(si.on_update)
                    )
                    changed = True
                    n_fixed += 1
                out.append(inst)
            if changed:
                blk.instructions = out
    return n_fixed


def _build_nc(pix=PIX, split_waits=True):
    import concourse.bass as bass
    import concourse.mybir as mybir
    import concourse.tile as tile

    f32 = mybir.dt.float32
    f16 = mybir.dt.float16
    nblk = pix // BLK
    ngrp = M // GROUP        # 32 groups total
    cgrp = MC // GROUP       # 4 groups per m-chunk
    GB = 4                   # pixel-blocks per Z-accumulation super-group
    nc = bass.Bass()
    # fp16 matmul chain: PE streams 1 cyc/row (fp32 is 4 cyc) and with this
    # problem's O(1) value ranges fp16's 10-bit mantissa keeps the output
    # within ~1e-3 relative of the fp32 reference.
    x_in = nc.declare_dram_parameter("x_bf", [D, pix], f16, isOutput=False)
    w_in = nc.declare_dram_parameter("normals_bf", [D, M], f16, isOutput=False)
    # gsum_w[:, mc*32:(mc+1)*32]: [128, 32] indicator, [p, r] = 1 iff
    # r == 8*(p//32) + replica; row r of the product = group (p//32) sum,
    # replicated so mm3 can select it from any 32-aligned base.
    g_in = nc.declare_dram_parameter("gsum_w", [MC, NCHUNK * ngrp], f16,
                                     isOutput=False)
    # gbc_w[32*q + r, mc*128 + p]: replica q of the [32, 128] indicator that
    # broadcasts group row r onto chunk partitions p; replicas let mm3's
    # stationary operand start at the same partition base as its moving
    # operand (rows 32*bg of the shared Z tile).
    b_in = nc.declare_dram_parameter("gbc_w", [GB * ngrp, NCHUNK * MC], f16,
                                     isOutput=False)
    # bias_neg[p, mc] = -<normals, offsets> for m = mc*128+p; applied as the
    # ACT exp per-partition bias (exact fp32, frees the K=65 aug row).
    c_in = nc.declare_dram_parameter("bias_neg", [MC, NCHUNK], f32,
                                     isOutput=False)
    y_out = nc.declare_dram_parameter("y", [M, pix], f32, isOutput=True)

    with tile.TileContext(nc) as tc:
        with (
            tc.tile_pool(name="const", bufs=1) as cpool,
            tc.tile_pool(name="xin", bufs=3) as xpool,
            tc.tile_pool(name="expv", bufs=GB * NCHUNK + 20) as epool,
            tc.tile_pool(name="zsb", bufs=2) as zpool,
            tc.tile_pool(name="recv", bufs=2) as rpool,
            tc.tile_pool(name="outv", bufs=4) as opool,
            tc.tile_pool(name="psl", bufs=2, space="PSUM") as psl,
            tc.tile_pool(name="psz", bufs=1, space="PSUM") as psz,
            tc.tile_pool(name="psb", bufs=2, space="PSUM") as psb,
        ):
            w_sb = cpool.tile([D, M], f16)
            nc.sync.dma_start(w_sb[:], w_in[:])
            g_sb = cpool.tile([MC, NCHUNK * ngrp], f16)
            nc.sync.dma_start(g_sb[:], g_in[:])
            b_sb = cpool.tile([GB * ngrp, NCHUNK * MC], f16)
            nc.sync.dma_start(b_sb[:], b_in[:])
            c_sb = cpool.tile([MC, NCHUNK], f32)
            nc.sync.dma_start(c_sb[:], c_in[:])

            ngroups = nblk // GB
            e_all = {}
            rec_of = {}

            x_of = {}

            def fetch_x(blk):
                if blk in x_of or blk >= pix // BLK:
                    return
                x_t = xpool.tile([D, BLK], f16, tag="x_t", name="x_t")
                nc.sync.dma_start(x_t[:], x_in[:, blk * BLK:(blk + 1) * BLK])
                x_of[blk] = x_t

            def phase1(g, bg, mc):
                blk = g * GB + bg
                if mc == 0:
                    fetch_x(blk)
                if mc == 2:
                    # prefetch the next block's x so the first matmuls of
                    # that block never wait on the DMA
                    fetch_x(blk + 1)
                x_t = x_of[blk]
                ps_l = psl.tile([MC, BLK], f32, tag="ps_l", name="ps_l")
                for h in range(BLK // FMAX):
                    nc.tensor.matmul(
                        ps_l[:, h * FMAX:(h + 1) * FMAX],
                        w_sb[:, mc * MC:(mc + 1) * MC],
                        x_t[:, h * FMAX:(h + 1) * FMAX],
                        start=True, stop=True,
                    )
                e_t = epool.tile([MC, BLK], f16, tag="e_t", name="e_t")
                nc.scalar.activation(
                    e_t[:], ps_l[:],
                    mybir.ActivationFunctionType.Exp,
                    bias=c_sb[:, mc:mc + 1],
                )
                e_all[g, bg, mc] = e_t
                ps_z = e_all["z", g]
                for h in range(BLK // FMAX):
                    nc.tensor.matmul(
                        ps_z[bg * ngrp:(bg + 1) * ngrp,
                             h * FMAX:(h + 1) * FMAX],
                        g_sb[:, mc * ngrp:(mc + 1) * ngrp],
                        e_t[:, h * FMAX:(h + 1) * FMAX],
                        start=(mc == 0), stop=(mc == NCHUNK - 1),
                        tile_position=(0, bg * ngrp),
                    )

            z_of = {}

            def copy_z(g):
                # free the psz tile quickly (ACT copy shares the exp table)
                # so the next group's Z accumulation can start
                ps_z = e_all.pop(("z", g))
                z_sb = zpool.tile([GB * ngrp, BLK], f32, tag="z_sb", name="z_sb")
                nc.scalar.copy(z_sb[:], ps_z[:])
                z_of[g] = z_sb

            def emit_recip(g, q):
                # emitted later, as column-quarters spaced across units, so
                # the in-order DVE stream interleaves multiplies between the
                # reciprocal pieces instead of stalling 6.5us behind one
                # monolithic instruction (reciprocal cost is free-dim driven,
                # so column splits keep the total constant)
                QW = BLK // 4
                if q == 0:
                    z_of[g, "rec"] = rpool.tile(
                        [GB * ngrp, BLK], f16, tag="rec", name="rec"
                    )
                rec = z_of[g, "rec"]
                z_sb = z_of[g]
                with nc.allow_low_precision(
                    reason="fp16 rounding of reciprocal feeding the "
                    "broadcast matmul; well within output tolerance"
                ):
                    nc.vector.reciprocal(
                        rec[:, q * QW:(q + 1) * QW],
                        z_sb[:, q * QW:(q + 1) * QW],
                    )
                if q == 3:
                    del z_of[g]
                    rec_of[g] = z_of.pop((g, "rec"))

            def phase2(g, bg, mc):
                blk = g * GB + bg
                rec = rec_of[g]
                e_t = e_all.pop((g, bg, mc))
                o_t = opool.tile([MC, BLK], f32, tag="o_t", name="o_t")
                for h in range(BLK // FMAX):
                    ps_b = psb.tile([MC, FMAX], f32, tag="ps_b", name="ps_b")
                    nc.tensor.matmul(
                        ps_b[:],
                        b_sb[bg * ngrp:(bg + 1) * ngrp,
                             mc * MC:(mc + 1) * MC],
                        rec[bg * ngrp:(bg + 1) * ngrp,
                            h * FMAX:(h + 1) * FMAX],
                        start=True, stop=True,
                        tile_position=(bg * ngrp, 0),
                    )
                    nc.vector.tensor_mul(
                        o_t[:, h * FMAX:(h + 1) * FMAX],
                        e_t[:, h * FMAX:(h + 1) * FMAX],
                        ps_b[:],
                    )
                nc.sync.dma_start(
                    y_out[mc * MC:(mc + 1) * MC,
                          blk * BLK:(blk + 1) * BLK],
                    o_t[:],
                )

            # Software pipeline: phase2 lags phase1 by one full group plus
            # LAG chunks, so the copy+reciprocal chain finishes while the PE
            # streams the next group's matmuls instead of stalling on it.
            CPG = GB * NCHUNK            # chunks per group
            LAG = 14
            FZLAG = 4
            nunits = ngroups * CPG
            recip_at = {}

            def unit(j):
                return j // CPG, (j // NCHUNK) % GB, j % NCHUNK

            for j in range(nunits + CPG + LAG):
                if j < nunits:
                    g, bg, mc = unit(j)
                    if bg == 0 and mc == 0:
                        e_all["z", g] = psz.tile(
                            [GB * ngrp, BLK], f32, tag="ps_z", name="ps_z"
                        )
                    phase1(g, bg, mc)
                    if bg == GB - 1 and mc == NCHUNK - 1:
                        copy_z(g)
                        for q in range(4):
                            recip_at[j + FZLAG + 2 * q] = (g, q)
                if j in recip_at:
                    emit_recip(*recip_at.pop(j))
                k = j - CPG - LAG
                if k >= 0:
                    phase2(*unit(k))
    if split_waits:
        _split_excess_waits(nc)
    return nc


def _prep_core_inputs(x, normals, offsets, sibmat):
    bf16 = np.float16
    bias = np.einsum("md,md->m", normals, offsets).astype(np.float32)
    w_bf = np.ascontiguousarray(normals.T).astype(bf16)

    ngrp = M // GROUP
    gid = np.arange(M) // GROUP                     # global group of each m
    gsum = np.zeros((MC, NCHUNK * ngrp), np.float32)
    gbc = np.zeros((ngrp, NCHUNK * MC), np.float32)
    for mc in range(NCHUNK):
        for p in range(MC):
            r = gid[mc * MC + p]
            gsum[p, mc * ngrp + r] = 1.0
            gbc[r, mc * MC + p] = 1.0
    gsum = gsum.astype(bf16)
    gbc = np.tile(gbc, (4, 1)).astype(bf16)
    bias_neg = np.ascontiguousarray(
        -bias.reshape(NCHUNK, MC).T
    ).astype(np.float32)

    in_maps = []
    for b in range(NCORES):
        x_bf = np.ascontiguousarray(x[b].reshape(D, PIX)).astype(bf16)
        in_maps.append(
            {"x_bf": x_bf, "normals_bf": w_bf, "gsum_w": gsum,
             "gbc_w": gbc, "bias_neg": bias_neg}
        )
    return in_maps


def _sibmat_is_expected(sibmat):
    gid = np.arange(M) // GROUP
    expected = (gid[:, None] == gid[None, :]).astype(np.float32)
    return sibmat.shape == (M, M) and np.array_equal(sibmat, expected)


def _numpy_fallback(x, normals, offsets, sibmat):
    # Straight fp32 transcription of the reference; only used if sibmat is
    # not the expected 32-wide block-diagonal matrix.
    bias = np.einsum("md,md->m", normals, offsets)
    out = np.empty((B, M, H, W), np.float32)
    for b in range(B):
        logits = np.einsum("dhw,md->mhw", x[b], normals) - bias[:, None, None]
        logits -= np.max(logits, axis=0, keepdims=True)
        e = np.exp(logits)
        z = np.einsum("mhw,nm->nhw", e, sibmat)
        out[b] = e / (z + 1e-15)
    return out


def kernel(x, normals, offsets, sibmat, steps=None, trace=False, **_ignored):
    from concourse.bass_utils import run_bass_kernel_spmd


    x = np.ascontiguousarray(np.asarray(x, dtype=np.float32))
    normals = np.ascontiguousarray(np.asarray(normals, dtype=np.float32))
    offsets = np.ascontiguousarray(np.asarray(offsets, dtype=np.float32))
    sibmat = np.ascontiguousarray(np.asarray(sibmat, dtype=np.float32))

    if (
        not _sibmat_is_expected(sibmat)
        or np.abs(normals).max() > 0.5
        or np.abs(x).max() > 12.0
    ):
        # unexpected structure or value ranges outside the fp16-safe
        # envelope of the device kernel: compute exactly on host
        return _numpy_fallback(x, normals, offsets, sibmat)

    if "nc" not in _cache:
        _cache["nc"] = _build_nc()
    nc = _cache["nc"]

    in_maps = _prep_core_inputs(x, normals, offsets, sibmat)
    res = run_bass_kernel_spmd(nc, in_maps, list(range(NCORES)), trace=trace)
    out = np.stack(
        [np.asarray(r["y"]).reshape(M, H, W) for r in res.results]
    )
    kernel.last_result = res
    return out


# revision 39
# speedup vs baseline: 1.2074x; 1.0059x over previous
"""Trainium2 Bass kernel for hierarchical (sibling-group) softmax over
hyperplane margins.

For x:(8,64,128,128), normals/offsets:(1024,64), sibmat block-diagonal with
32-wide sibling groups:

    logits[b,m,h,w] = <x[b,:,h,w], normals[m]> - <normals[m], offsets[m]>
    out = exp(logits) / (group_sum_32(exp(logits)) + 1e-15)

Sharding: data-parallel over batch, one batch element per NeuronCore (8
cores), no collectives.  Per core, m-chunks of 128 rows live on partitions
and pixels on the free axis, in blocks of 1024 pixels:

  1. PE matmul (fp16 in, fp32 psum): logits = normals.T @ x.
  2. ACT exp PSUM->SBUF with the per-partition bias argument supplying
     -<normals_m, offsets_m> exactly in fp32.
  3. PE matmul with a [128, 32] group-indicator accumulates the 32-wide
     group sums of four pixel blocks into one base-aligned [128, 1024]
     PSUM tile (rows 32*bg + group).
  4. ACT copies Z to SBUF (shares the exp ACT table), one DVE reciprocal
     per 4-block super-group (the DVE reciprocal is ~6.3 cyc/elem, so it
     must be amortized), output in fp16.
  5. PE matmul with a [32, 128] indicator broadcasts 1/Z rows back onto
     the chunk's 128 partitions (PSUM), and one DVE multiply per half
     produces the fp32 output tile.
  6. 512KB output DMAs land directly in the reference (M, H, W) layout.

Phase 5/6 of each super-group is software-pipelined ~1.4 groups behind
phases 1-3 so the reciprocal latency hides under PE work.  A post-pass
splits multi-wait instructions (walrus's TRN2 codegen encodes at most one
semaphore wait per compute instruction).  Matmul operands are fp16 because
fp32 matmuls stream at 4 cyc/row and the PE clock holds at 1.2 GHz in this
dependency pattern; all value ranges here are O(1) so fp16's 10-bit
mantissa keeps the output within ~1.5e-3 of the fp32 reference (guarded by
input-range checks that fall back to an exact host implementation).
"""

import numpy as np

B, D, H, W = 8, 64, 128, 128
M = 1024
GROUP = 32
PIX = H * W          # 16384 pixels per batch element
BLK = 1024           # pixels per block
NBLK = PIX // BLK    # 16
MC = 128             # m-chunk width (partition dim)
NCHUNK = M // MC     # 8
KAUG = D + 1         # contraction dim augmented with the bias row
NCORES = 8
FMAX = 512           # max moving free dim for fp32 matmul

_cache = {}


_WAIT_OK_OPCODES = {"Call"}


def _split_excess_waits(nc):
    """Walrus's TRN2 codegen (CoreV3GenImpl setupSyncWait) encodes at most
    one semaphore wait per compute instruction (Matmult, TensorTensor, ...);
    Tile can legitimately attach several (e.g. waits on two input DMAs).
    Move all but one wait onto EventSemaphore instructions inserted just
    before the instruction on the same engine — ordering is identical."""
    import concourse.mybir as mybir

    n_fixed = 0
    for f in nc.m.functions:
        for blk in f.blocks:
            out = []
            changed = False
            for inst in blk.instructions:
                si = inst.sync_info
                if (
                    si is not None
                    and len(si.on_wait) > 1
                    and inst.opcode not in _WAIT_OK_OPCODES
                ):
                    waits = list(si.on_wait)
                    for j, w in enumerate(waits[:-1]):
                        out.append(
                            mybir.InstEventSemaphore(
                                name=f"{inst.name}-wsplit{j}",
                                opcode="EventSemaphore",
                                engine=inst.engine,
                                sync_info=mybir.SyncInfo(
                                    on_wait=[w], on_update=[]
                                ),
                            )
                        )
                    inst.sync_info = mybir.SyncInfo(
                        on_wait=[waits[-1]], on_update=list(si.on_update)
                    )
                    changed = True
                    n_fixed += 1
                out.append(inst)
            if changed:
                blk.instructions = out
    return n_fixed


def _build_nc(pix=PIX, split_waits=True):
    import concourse.bass as bass
    import concourse.mybir as mybir
    import concourse.tile as tile

    f32 = mybir.dt.float32
    f16 = mybir.dt.float16
    nblk = pix // BLK
    ngrp = M // GROUP        # 32 groups total
    cgrp = MC // GROUP       # 4 groups per m-chunk
    GB = 4                   # pixel-blocks per Z-accumulation super-group
    nc = bass.Bass()
    # fp16 matmul chain: PE streams 1 cyc/row (fp32 is 4 cyc) and with this
    # problem's O(1) value ranges fp16's 10-bit mantissa keeps the output
    # within ~1e-3 relative of the fp32 reference.
    x_in = nc.declare_dram_parameter("x_bf", [D, pix], f16, isOutput=False)
    w_in = nc.declare_dram_parameter("normals_bf", [D, M], f16, isOutput=False)
    # gsum_w[:, mc*32:(mc+1)*32]: [128, 32] indicator, [p, r] = 1 iff
    # r == 8*(p//32) + replica; row r of the product = group (p//32) sum,
    # replicated so mm3 can select it from any 32-aligned base.
    g_in = nc.declare_dram_parameter("gsum_w", [MC, NCHUNK * ngrp], f16,
                                     isOutput=False)
    # gbc_w[32*q + r, mc*128 + p]: replica q of the [32, 128] indicator that
    # broadcasts group row r onto chunk partitions p; replicas let mm3's
    # stationary operand start at the same partition base as its moving
    # operand (rows 32*bg of the shared Z tile).
    b_in = nc.declare_dram_parameter("gbc_w", [GB * ngrp, NCHUNK * MC], f16,
                                     isOutput=False)
    # bias_neg[p, mc] = -<normals, offsets> for m = mc*128+p; applied as the
    # ACT exp per-partition bias (exact fp32, frees the K=65 aug row).
    c_in = nc.declare_dram_parameter("bias_neg", [MC, NCHUNK], f32,
                                     isOutput=False)
    y_out = nc.declare_dram_parameter("y", [M, pix], f32, isOutput=True)

    with tile.TileContext(nc) as tc:
        with (
            tc.tile_pool(name="const", bufs=1) as cpool,
            tc.tile_pool(name="xin", bufs=4) as xpool,
            tc.tile_pool(name="expv", bufs=GB * NCHUNK + 20) as epool,
            tc.tile_pool(name="zsb", bufs=2) as zpool,
            tc.tile_pool(name="recv", bufs=2) as rpool,
            tc.tile_pool(name="outv", bufs=6) as opool,
            tc.tile_pool(name="psl", bufs=2, space="PSUM") as psl,
            tc.tile_pool(name="psz", bufs=1, space="PSUM") as psz,
            tc.tile_pool(name="psb", bufs=2, space="PSUM") as psb,
        ):
            w_sb = cpool.tile([D, M], f16)
            nc.sync.dma_start(w_sb[:], w_in[:])
            g_sb = cpool.tile([MC, NCHUNK * ngrp], f16)
            nc.sync.dma_start(g_sb[:], g_in[:])
            b_sb = cpool.tile([GB * ngrp, NCHUNK * MC], f16)
            nc.sync.dma_start(b_sb[:], b_in[:])
            c_sb = cpool.tile([MC, NCHUNK], f32)
            nc.sync.dma_start(c_sb[:], c_in[:])

            ngroups = nblk // GB
            e_all = {}
            rec_of = {}

            x_of = {}

            def fetch_x(blk):
                if blk in x_of or blk >= pix // BLK:
                    return
                x_t = xpool.tile([D, BLK], f16, tag="x_t", name="x_t")
                nc.sync.dma_start(x_t[:], x_in[:, blk * BLK:(blk + 1) * BLK])
                x_of[blk] = x_t

            def phase1(g, bg, mc):
                blk = g * GB + bg
                if mc == 0:
                    fetch_x(blk)
                if mc == 2:
                    # prefetch the next block's x so the first matmuls of
                    # that block never wait on the DMA
                    fetch_x(blk + 1)
                x_t = x_of[blk]
                ps_l = psl.tile([MC, BLK], f32, tag="ps_l", name="ps_l")
                for h in range(BLK // FMAX):
                    nc.tensor.matmul(
                        ps_l[:, h * FMAX:(h + 1) * FMAX],
                        w_sb[:, mc * MC:(mc + 1) * MC],
                        x_t[:, h * FMAX:(h + 1) * FMAX],
                        start=True, stop=True,
                    )
                e_t = epool.tile([MC, BLK], f16, tag="e_t", name="e_t")
                nc.scalar.activation(
                    e_t[:], ps_l[:],
                    mybir.ActivationFunctionType.Exp,
                    bias=c_sb[:, mc:mc + 1],
                )
                e_all[g, bg, mc] = e_t
                ps_z = e_all["z", g]
                for h in range(BLK // FMAX):
                    nc.tensor.matmul(
                        ps_z[bg * ngrp:(bg + 1) * ngrp,
                             h * FMAX:(h + 1) * FMAX],
                        g_sb[:, mc * ngrp:(mc + 1) * ngrp],
                        e_t[:, h * FMAX:(h + 1) * FMAX],
                        start=(mc == 0), stop=(mc == NCHUNK - 1),
                        tile_position=(0, bg * ngrp),
                    )

            z_of = {}

            def copy_z(g):
                # free the psz tile quickly (ACT copy shares the exp table)
                # so the next group's Z accumulation can start
                ps_z = e_all.pop(("z", g))
                z_sb = zpool.tile([GB * ngrp, BLK], f32, tag="z_sb", name="z_sb")
                nc.scalar.copy(z_sb[:], ps_z[:])
                z_of[g] = z_sb

            def emit_recip(g, q):
                # emitted later, as column-quarters spaced across units, so
                # the in-order DVE stream interleaves multiplies between the
                # reciprocal pieces instead of stalling 6.5us behind one
                # monolithic instruction (reciprocal cost is free-dim driven,
                # so column splits keep the total constant)
                QW = BLK // 4
                if q == 0:
                    z_of[g, "rec"] = rpool.tile(
                        [GB * ngrp, BLK], f16, tag="rec", name="rec"
                    )
                rec = z_of[g, "rec"]
                z_sb = z_of[g]
                with nc.allow_low_precision(
                    reason="fp16 rounding of reciprocal feeding the "
                    "broadcast matmul; well within output tolerance"
                ):
                    nc.vector.reciprocal(
                        rec[:, q * QW:(q + 1) * QW],
                        z_sb[:, q * QW:(q + 1) * QW],
                    )
                if q == 3:
                    del z_of[g]
                    rec_of[g] = z_of.pop((g, "rec"))

            def phase2(g, bg, mc):
                blk = g * GB + bg
                rec = rec_of[g]
                e_t = e_all.pop((g, bg, mc))
                o_t = opool.tile([MC, BLK], f32, tag="o_t", name="o_t")
                for h in range(BLK // FMAX):
                    ps_b = psb.tile([MC, FMAX], f32, tag="ps_b", name="ps_b")
                    nc.tensor.matmul(
                        ps_b[:],
                        b_sb[bg * ngrp:(bg + 1) * ngrp,
                             mc * MC:(mc + 1) * MC],
                        rec[bg * ngrp:(bg + 1) * ngrp,
                            h * FMAX:(h + 1) * FMAX],
                        start=True, stop=True,
                        tile_position=(bg * ngrp, 0),
                    )
                    nc.vector.tensor_mul(
                        o_t[:, h * FMAX:(h + 1) * FMAX],
                        e_t[:, h * FMAX:(h + 1) * FMAX],
                        ps_b[:],
                    )
                nc.sync.dma_start(
                    y_out[mc * MC:(mc + 1) * MC,
                          blk * BLK:(blk + 1) * BLK],
                    o_t[:],
                )

            # Software pipeline: phase2 lags phase1 by one full group plus
            # LAG chunks, so the copy+reciprocal chain finishes while the PE
            # streams the next group's matmuls instead of stalling on it.
            CPG = GB * NCHUNK            # chunks per group
            LAG = 14
            FZLAG = 4
            nunits = ngroups * CPG
            recip_at = {}

            def unit(j):
                return j // CPG, (j // NCHUNK) % GB, j % NCHUNK

            for j in range(nunits + CPG + LAG):
                if j < nunits:
                    g, bg, mc = unit(j)
                    if bg == 0 and mc == 0:
                        e_all["z", g] = psz.tile(
                            [GB * ngrp, BLK], f32, tag="ps_z", name="ps_z"
                        )
                    phase1(g, bg, mc)
                    if bg == GB - 1 and mc == NCHUNK - 1:
                        copy_z(g)
                        for q in range(4):
                            recip_at[j + FZLAG + 2 * q] = (g, q)
                if j in recip_at:
                    emit_recip(*recip_at.pop(j))
                k = j - CPG - LAG
                if k >= 0:
                    phase2(*unit(k))
    if split_waits:
        _split_excess_waits(nc)
    return nc


def _prep_core_inputs(x, normals, offsets, sibmat):
    bf16 = np.float16
    bias = np.einsum("md,md->m", normals, offsets).astype(np.float32)
    w_bf = np.ascontiguousarray(normals.T).astype(bf16)

    ngrp = M // GROUP
    gid = np.arange(M) // GROUP                     # global group of each m
    gsum = np.zeros((MC, NCHUNK * ngrp), np.float32)
    gbc = np.zeros((ngrp, NCHUNK * MC), np.float32)
    for mc in range(NCHUNK):
        for p in range(MC):
            r = gid[mc * MC + p]
            gsum[p, mc * ngrp + r] = 1.0
            gbc[r, mc * MC + p] = 1.0
    gsum = gsum.astype(bf16)
    gbc = np.tile(gbc, (4, 1)).astype(bf16)
    bias_neg = np.ascontiguousarray(
        -bias.reshape(NCHUNK, MC).T
    ).astype(np.float32)

    in_maps = []
    for b in range(NCORES):
        x_bf = np.ascontiguousarray(x[b].reshape(D, PIX)).astype(bf16)
        in_maps.append(
            {"x_bf": x_bf, "normals_bf": w_bf, "gsum_w": gsum,
             "gbc_w": gbc, "bias_neg": bias_neg}
        )
    return in_maps


def _sibmat_is_expected(sibmat):
    gid = np.arange(M) // GROUP
    expected = (gid[:, None] == gid[None, :]).astype(np.float32)
    return sibmat.shape == (M, M) and np.array_equal(sibmat, expected)


def _numpy_fallback(x, normals, offsets, sibmat):
    # Straight fp32 transcription of the reference; only used if sibmat is
    # not the expected 32-wide block-diagonal matrix.
    bias = np.einsum("md,md->m", normals, offsets)
    out = np.empty((B, M, H, W), np.float32)
    for b in range(B):
        logits = np.einsum("dhw,md->mhw", x[b], normals) - bias[:, None, None]
        logits -= np.max(logits, axis=0, keepdims=True)
        e = np.exp(logits)
        z = np.einsum("mhw,nm->nhw", e, sibmat)
        out[b] = e / (z + 1e-15)
    return out


def kernel(x, normals, offsets, sibmat, steps=None, trace=False, **_ignored):
    from concourse.bass_utils import run_bass_kernel_spmd


    x = np.ascontiguousarray(np.asarray(x, dtype=np.float32))
    normals = np.ascontiguousarray(np.asarray(normals, dtype=np.float32))
    offsets = np.ascontiguousarray(np.asarray(offsets, dtype=np.float32))
    sibmat = np.ascontiguousarray(np.asarray(sibmat, dtype=np.float32))

    if (
        not _sibmat_is_expected(sibmat)
        or np.abs(normals).max() > 0.5
        or np.abs(x).max() > 12.0
    ):
        # unexpected structure or value ranges outside the fp16-safe
        # envelope of the device kernel: compute exactly on host
        return _numpy_fallback(x, normals, offsets, sibmat)

    if "nc" not in _cache:
        _cache["nc"] = _build_nc()
    nc = _cache["nc"]

    in_maps = _prep_core_inputs(x, normals, offsets, sibmat)
    res = run_bass_kernel_spmd(nc, in_maps, list(range(NCORES)), trace=trace)
    out = np.stack(
        [np.asarray(r["y"]).reshape(M, H, W) for r in res.results]
    )
    kernel.last_result = res
    return out
